# revision 1
# baseline (speedup 1.0000x reference)
"""Al-Salam-Carlitz KAN layer on 8 TRN2 NeuronCores.

Math: y[b,o] = sum_{i,d} P_d(tanh(x[b,i])) * coeffs[i,o,d], where P_d are the
Al-Salam-Carlitz polynomials given by a three-term recurrence in scalars a, q.
Each P_d is a degree-d polynomial in t = tanh(x), so on the host we fold the
(D+1)x(D+1) basis-change matrix into coeffs:

    y[b,o] = bias[o] + sum_{k=1..D} sum_i t[b,i]^k * Cf[i,o,k]

with bias[o] = sum_i Cf[i,o,0] (the k=0 column times t^0 == 1).  This removes
1/8 of the matmul work and leaves the device with: tanh, a bf16 power chain,
and a K=7*1024 contraction done as 448 TensorE matmuls per core.

Sharding: data-parallel over batch (4096 -> 8 x 512).  Each core receives its
x-shard pre-transposed ([I, 512], so the contraction dim lands on SBUF
partitions), the folded weights (bf16, pre-laid-out for contiguous DMA), and
the bias.  No collectives; the host concatenates the 8 output shards.
"""

import numpy as np
import ml_dtypes

B, I, O, D1 = 4096, 1024, 1024, 8
NCORES = 8
BS = B // NCORES       # batch rows per core (moving free dim of each matmul)
IC = I // 128          # i chunks (contraction tiles per power plane)
OC = O // 128          # o chunks (output partition tiles)
NK = D1 - 1            # power planes k = 1..7
NJ = IC * NK           # contraction steps per output tile

_GRAPH = None
LAST_RESULT = None     # BassKernelResults of the most recent run (for test.py)


def _build_graph():
    import concourse.tile as tile
    from concourse import bacc, mybir

    nc = bacc.Bacc("TRN2", target_bir_lowering=False, debug=False,
                   num_devices=NCORES)
    f32 = mybir.dt.float32
    bf16 = mybir.dt.bfloat16

    xT = nc.dram_tensor("xT", [I, BS], f32, kind="ExternalInput").ap()
    cw = nc.dram_tensor("cw", [OC, 128, NJ * 128], bf16,
                        kind="ExternalInput").ap()
    bias = nc.dram_tensor("bias", [128, OC], f32, kind="ExternalInput").ap()
    yT = nc.dram_tensor("yT", [O, BS], f32, kind="ExternalOutput").ap()

    with tile.TileContext(nc) as tc:
        with tc.tile_pool(name="xin", bufs=3) as xin_pool, \
             tc.tile_pool(name="planes", bufs=NJ) as plane_pool, \
             tc.tile_pool(name="cwp", bufs=3) as cw_pool, \
             tc.tile_pool(name="misc", bufs=1) as misc_pool, \
             tc.tile_pool(name="psum", bufs=2, space="PSUM") as psum_pool, \
             tc.tile_pool(name="osb", bufs=2) as out_pool:

            bias_t = misc_pool.tile([128, OC], f32, tag="bias")
            nc.sync.dma_start(bias_t[:], bias[:])

            # power planes t^k, k=1..7, for each i-chunk; all stay resident
            planes = []
            for ic in range(IC):
                xin = xin_pool.tile([128, BS], f32, tag="xin")
                nc.sync.dma_start(xin[:], xT[ic * 128:(ic + 1) * 128, :])
                xt = plane_pool.tile([128, BS], bf16, tag="planes", name="xt")
                nc.scalar.activation(xt[:], xin[:],
                                     mybir.ActivationFunctionType.Tanh)
                planes.append(xt)
                prev = xt
                for k in range(2, D1):
                    pw = plane_pool.tile([128, BS], bf16, tag="planes",
                                         name="pw")
                    nc.vector.tensor_mul(pw[:], prev[:], xt[:])
                    planes.append(pw)
                    prev = pw

            # one output tile [128 o, BS b] per o-chunk; K-contiguous matmuls
            for oc in range(OC):
                cwt = cw_pool.tile([128, NJ * 128], bf16, tag="cw", name="cwt")
                nc.sync.dma_start(cwt[:], cw[oc])
                ps = psum_pool.tile([128, BS], f32, tag="ps", name="ps")
                for j in range(NJ):
                    nc.tensor.matmul(ps[:], cwt[:, j * 128:(j + 1) * 128],
                                     planes[j][:],
                                     start=(j == 0), stop=(j == NJ - 1))
                ot = out_pool.tile([128, BS], f32, tag="ot", name="ot")
                nc.scalar.activation(ot[:], ps[:],
                                     mybir.ActivationFunctionType.Identity,
                                     bias=bias_t[:, oc:oc + 1])
                nc.sync.dma_start(yT[oc * 128:(oc + 1) * 128, :], ot[:])

    nc.compile()
    return nc


def _get_graph():
    global _GRAPH
    if _GRAPH is None:
        _GRAPH = _build_graph()
    return _GRAPH


def _host_prep(a, q, coeffs):
    """Fold the polynomial basis change into the weights (float64 on host)."""
    # c[d, k]: P_d(t) = sum_k c[d, k] * t^k, from the three-term recurrence
    c = np.zeros((D1, D1), np.float64)
    c[0, 0] = 1.0
    if D1 > 1:
        c[1, 1] = 1.0
        c[1, 0] = -a
    for n in range(2, D1):
        c[n, 1:] += c[n - 1, :-1]
        c[n, :] -= (a + q ** n) * c[n - 1, :]
        c[n, :] -= a * q ** (n - 1) * c[n - 2, :]

    Cf = (coeffs.reshape(-1, D1).astype(np.float64) @ c).reshape(I, O, D1)
    bias = Cf[:, :, 0].sum(axis=0).astype(np.float32)                # [O]
    Ck = Cf[:, :, 1:].astype(np.float32).astype(ml_dtypes.bfloat16)  # [I,O,NK]

    # device layout: cw[oc, p, (ic*NK + (k-1))*128 + ol] = Ck[ic*128+p, oc*128+ol, k-1]
    t = Ck.reshape(IC, 128, OC, 128, NK)            # [ic, p, oc, ol, k-1]
    cw_dev = np.ascontiguousarray(
        t.transpose(2, 1, 0, 4, 3)).reshape(OC, 128, NJ * 128)
    bias_dev = np.ascontiguousarray(bias.reshape(OC, 128).T)  # [128, OC]
    return cw_dev, bias_dev


def kernel(x, a, q, coeffs):
    global LAST_RESULT
    from concourse.bass_utils import run_bass_kernel_spmd

    x = np.ascontiguousarray(np.asarray(x, dtype=np.float32))
    coeffs = np.ascontiguousarray(np.asarray(coeffs, dtype=np.float32))
    a_val = float(np.asarray(a).reshape(-1)[0])
    q_val = float(np.asarray(q).reshape(-1)[0])

    cw_dev, bias_dev = _host_prep(a_val, q_val, coeffs)
    xs = x.reshape(NCORES, BS, I).transpose(0, 2, 1)  # [core, I, BS]

    in_maps = [{
        "xT": np.ascontiguousarray(xs[c]),
        "cw": cw_dev,
        "bias": bias_dev,
    } for c in range(NCORES)]

    nc = _get_graph()
    res = run_bass_kernel_spmd(nc, in_maps, core_ids=list(range(NCORES)))
    LAST_RESULT = res

    shards = [np.asarray(res.results[c]["yT"]).T for c in range(NCORES)]
    return np.ascontiguousarray(np.concatenate(shards, axis=0),
                                dtype=np.float32)


if __name__ == "__main__":
    rng = np.random.default_rng(0)
    inputs = {
        "x": rng.standard_normal((B, I), dtype=np.float32),
        "a": np.zeros((1,), np.float32),
        "q": np.ones((1,), np.float32),
        "coeffs": rng.standard_normal((I, O, D1), dtype=np.float32)
        / (I * D1),
    }
    y = kernel(**inputs)
    print("out", y.shape, y.dtype, float(np.abs(y).mean()))


# revision 2
# speedup vs baseline: 1.0284x; 1.0284x over previous
"""Al-Salam-Carlitz KAN layer on 8 TRN2 NeuronCores.

Math: y[b,o] = sum_{i,d} P_d(tanh(x[b,i])) * coeffs[i,o,d], where P_d are the
Al-Salam-Carlitz polynomials given by a three-term recurrence in scalars a, q.
Each P_d is a degree-d polynomial in t = tanh(x), so on the host we fold the
(D+1)x(D+1) basis-change matrix into coeffs:

    y[b,o] = bias[o] + sum_{k=1..D} sum_i t[b,i]^k * Cf[i,o,k]

with bias[o] = sum_i Cf[i,o,0] (the k=0 column times t^0 == 1).  This removes
1/8 of the matmul work and leaves the device with: tanh, a bf16 power chain,
and a K=7*1024 contraction done as 448 TensorE matmuls per core.

Sharding: data-parallel over batch (4096 -> 8 x 512).  Each core receives its
x-shard pre-transposed ([I, 512], so the contraction dim lands on SBUF
partitions), the folded weights (bf16, pre-laid-out in exact consumption
order for contiguous chunked DMA), and the bias.  No collectives; the host
concatenates the 8 output shards.

Matmul schedule (one core): 8 output tiles yT[oc] = [128 o, 512 b], each
accumulating 56 K-steps in PSUM bank oc.
  Phase A (j = 0..13): for each j, one matmul into every bank -- consumption
    of power planes is 8x slower than production, so the PE never stalls on
    the tanh/power chain during ramp-up.
  Phase B (oc = 0..7): finish each bank's remaining 42 K-steps back-to-back,
    so banks complete staggered and PSUM evacuation + output DMA overlap the
    next bank's matmuls.
"""

import numpy as np
import ml_dtypes

B, I, O, D1 = 4096, 1024, 1024, 8
NCORES = 8
BS = B // NCORES       # batch rows per core (moving free dim of each matmul)
IC = I // 128          # i chunks (contraction tiles per power plane)
OC = O // 128          # o chunks (output partition tiles)
NK = D1 - 1            # power planes k = 1..7
NJ = IC * NK           # K-steps per output tile
NJA = 14               # phase-A K-steps (covers planes of i-chunks 0..1)

# (oc, j) consumption order of the 448 stationary weight tiles
SEQ = [(oc, j) for j in range(NJA) for oc in range(OC)] + \
      [(oc, j) for oc in range(OC) for j in range(NJA, NJ)]
# chunk boundaries (tiles per weight DMA): phase A by j (8 tiles), phase B by
# i-chunk runs (7 tiles)
CHUNKS = [(j * OC, OC) for j in range(NJA)] + \
         [(NJA * OC + (oc * (NJ - NJA) + r * NK), NK)
          for oc in range(OC) for r in range((NJ - NJA) // NK)]
assert sum(c[1] for c in CHUNKS) == OC * NJ

_GRAPH = None
LAST_RESULT = None     # BassKernelResults of the most recent run (for test.py)


def _build_graph():
    import concourse.tile as tile
    from concourse import bacc, mybir

    nc = bacc.Bacc("TRN2", target_bir_lowering=False, debug=False,
                   num_devices=NCORES)
    f32 = mybir.dt.float32
    bf16 = mybir.dt.bfloat16

    xT = nc.dram_tensor("xT", [I, BS], f32, kind="ExternalInput").ap()
    cw = nc.dram_tensor("cw", [128, OC * NJ * 128], bf16,
                        kind="ExternalInput").ap()
    bias = nc.dram_tensor("bias", [128, OC], f32, kind="ExternalInput").ap()
    yT = nc.dram_tensor("yT", [O, BS], f32, kind="ExternalOutput").ap()

    with tile.TileContext(nc) as tc:
        with tc.tile_pool(name="xin", bufs=IC) as xin_pool, \
             tc.tile_pool(name="planes", bufs=NJ) as plane_pool, \
             tc.tile_pool(name="cwp", bufs=8) as cw_pool, \
             tc.tile_pool(name="misc", bufs=1) as misc_pool, \
             tc.tile_pool(name="psum", bufs=OC, space="PSUM") as psum_pool, \
             tc.tile_pool(name="osb", bufs=2) as out_pool:

            bias_t = misc_pool.tile([128, OC], f32, tag="bias")
            nc.sync.dma_start(bias_t[:], bias[:])

            # power planes t^k, k=1..7, per i-chunk; all stay resident.
            # DMA emission order (= sync-engine issue order): xin0, then the
            # first weight chunks interleaved with the remaining xins, then
            # the rest of the weight chunks — matches consumption order.
            planes = []
            cw_tiles = []

            def emit_cw_chunk(ci):
                s0, size = CHUNKS[ci]
                cwt = cw_pool.tile([128, size * 128], bf16, tag="cw",
                                   name="cwt")
                nc.sync.dma_start(cwt[:], cw[:, s0 * 128:(s0 + size) * 128])
                cw_tiles.append(cwt)

            for ic in range(IC):
                xin = xin_pool.tile([128, BS], f32, tag="xin", name="xin")
                nc.sync.dma_start(xin[:], xT[ic * 128:(ic + 1) * 128, :])
                xt = plane_pool.tile([128, BS], bf16, tag="planes", name="xt")
                nc.scalar.activation(xt[:], xin[:],
                                     mybir.ActivationFunctionType.Tanh)
                planes.append(xt)
                prev = xt
                for k in range(2, D1):
                    pw = plane_pool.tile([128, BS], bf16, tag="planes",
                                         name="pw")
                    nc.vector.tensor_mul(pw[:], prev[:], xt[:])
                    planes.append(pw)
                    prev = pw
                emit_cw_chunk(ic)  # first 8 weight chunks ride along

            ps_tiles = [psum_pool.tile([128, BS], f32, tag="ps", name="ps")
                        for _ in range(OC)]
            done = [0] * OC
            s = 0
            for ci, (s0, size) in enumerate(CHUNKS):
                if ci >= IC:
                    emit_cw_chunk(ci)
                cwt = cw_tiles[ci]
                for t in range(size):
                    oc, j = SEQ[s0 + t]
                    nc.tensor.matmul(ps_tiles[oc][:],
                                     cwt[:, t * 128:(t + 1) * 128],
                                     planes[j][:],
                                     start=(done[oc] == 0),
                                     stop=(done[oc] == NJ - 1))
                    done[oc] += 1
                    if done[oc] == NJ:
                        ot = out_pool.tile([128, BS], f32, tag="ot",
                                           name="ot")
                        nc.scalar.activation(
                            ot[:], ps_tiles[oc][:],
                            mybir.ActivationFunctionType.Identity,
                            bias=bias_t[:, oc:oc + 1])
                        nc.gpsimd.dma_start(
                            yT[oc * 128:(oc + 1) * 128, :], ot[:])
                    s += 1
            assert s == OC * NJ and all(d == NJ for d in done)

    nc.compile()
    return nc


def _get_graph():
    global _GRAPH
    if _GRAPH is None:
        _GRAPH = _build_graph()
    return _GRAPH


def _host_prep(a, q, coeffs):
    """Fold the polynomial basis change into the weights (float64 on host)."""
    # c[d, k]: P_d(t) = sum_k c[d, k] * t^k, from the three-term recurrence
    c = np.zeros((D1, D1), np.float64)
    c[0, 0] = 1.0
    if D1 > 1:
        c[1, 1] = 1.0
        c[1, 0] = -a
    for n in range(2, D1):
        c[n, 1:] += c[n - 1, :-1]
        c[n, :] -= (a + q ** n) * c[n - 1, :]
        c[n, :] -= a * q ** (n - 1) * c[n - 2, :]

    Cf = (coeffs.reshape(-1, D1).astype(np.float64) @ c).reshape(I, O, D1)
    bias = Cf[:, :, 0].sum(axis=0).astype(np.float32)                # [O]
    Ck = Cf[:, :, 1:].astype(np.float32).astype(ml_dtypes.bfloat16)  # [I,O,NK]

    # stationary tile for (oc, j=ic*NK+k1): [128 i-part, 128 o-col] slice
    t = Ck.reshape(IC, 128, OC, 128, NK)            # [ic, p, oc, ol, k1]
    X = np.ascontiguousarray(t.transpose(2, 0, 4, 1, 3)) \
          .reshape(OC, NJ, 128, 128)                # [oc, j, p, ol]
    oc_idx = np.array([oc for oc, _ in SEQ])
    j_idx = np.array([j for _, j in SEQ])
    seq_tiles = X[oc_idx, j_idx]                    # [448, p, ol]
    cw_dev = np.ascontiguousarray(
        seq_tiles.transpose(1, 0, 2)).reshape(128, OC * NJ * 128)
    bias_dev = np.ascontiguousarray(bias.reshape(OC, 128).T)  # [128, OC]
    return cw_dev, bias_dev


def kernel(x, a, q, coeffs):
    global LAST_RESULT
    from concourse.bass_utils import run_bass_kernel_spmd

    x = np.ascontiguousarray(np.asarray(x, dtype=np.float32))
    coeffs = np.ascontiguousarray(np.asarray(coeffs, dtype=np.float32))
    a_val = float(np.asarray(a).reshape(-1)[0])
    q_val = float(np.asarray(q).reshape(-1)[0])

    cw_dev, bias_dev = _host_prep(a_val, q_val, coeffs)
    xs = x.reshape(NCORES, BS, I).transpose(0, 2, 1)  # [core, I, BS]

    in_maps = [{
        "xT": np.ascontiguousarray(xs[c]),
        "cw": cw_dev,
        "bias": bias_dev,
    } for c in range(NCORES)]

    nc = _get_graph()
    res = run_bass_kernel_spmd(nc, in_maps, core_ids=list(range(NCORES)))
    LAST_RESULT = res

    shards = [np.asarray(res.results[c]["yT"]).T for c in range(NCORES)]
    return np.ascontiguousarray(np.concatenate(shards, axis=0),
                                dtype=np.float32)


if __name__ == "__main__":
    rng = np.random.default_rng(0)
    inputs = {
        "x": rng.standard_normal((B, I), dtype=np.float32),
        "a": np.zeros((1,), np.float32),
        "q": np.ones((1,), np.float32),
        "coeffs": rng.standard_normal((I, O, D1), dtype=np.float32)
        / (I * D1),
    }
    y = kernel(**inputs)
    print("out", y.shape, y.dtype, float(np.abs(y).mean()))
